# revision 9
# baseline (speedup 1.0000x reference)
"""Trainium2 Bass kernel for nn_CamFusionModule (epipolar max-sampling fusion).

Strategy
--------
Data-parallel over output pixels: the 64x64 heatmap grid is split into 8
row-bands of 8 rows, one per NeuronCore (heatmaps replicated, all 12
(curview, othview) pairs on every core, per the sharding hint's
"pair axis splittable / heatmaps replicated" guidance).

Host (numpy/jax-cpu, exact fp32, mirrors the reference op-for-op):
  * camera math -> per-pair epipolar line coords ysw (x-sweep) and
    xsh (y-sweep) for this core's 512 pixels; rint+clamp to [-1, 64]
    -> bf16 index rows ready to stream.
  * heatmaps -> per-(pair, sweep, t-pair) stationary tables, split into
    bf16 (hi, lo) parts laid out block-diagonally so one N=512 matmul
    pair gathers both t-parities for 16 channels.

Device (per NeuronCore), per (pair, sweep, t-pair):
  * PE outer-product matmul (K=2) replicates the two index rows across
    all 128 partitions into PSUM.
  * DVE `is_equal` against a per-partition iota (or ACT Square->Relu on a
    share of the work) turns that into a {0,1} one-hot mask [128, 512].
  * Two PE matmuls (hi, lo accumulated in PSUM) against the block-diag
    tables gather 2 samples/column x 16 channels; outputs of 4 t-pairs
    are stacked into one full-width PSUM bank via 32-aligned col groups.
  * DVE running tensor-max over PSUM banks, then partition-block folds
    (via small SBUF->SBUF shift DMAs) collapse t-pairs/parity/sweep.

Output: [12, 16, 512] fp32 per core, reassembled host-side.
"""

import numpy as np
import ml_dtypes

NVIEW = 4
B, C, H, W = 1, 16, 64, 64
HW = H * W
NPAIR = 12
NCORE = 8
PXS = HW // NCORE          # 512 pixels per core
ROWS = H // NCORE          # 8 image rows per core
NTP = W // 2               # 32 t-pairs per sweep
BIG = 1.0e9                # sentinel for non-finite coords (-> invalid)

_PAIRS = [(c, o) for c in range(NVIEW) for o in range(NVIEW) if o != c]


def _line_coords(affine_trans, cam_Intri, cam_R, cam_T, inv_affine_trans):
    """Mirror of the reference's fp32 math up to the raw sweep coordinates.

    Returns ysw[p, t, px], xsh[p, t, px] float32 arrays of shape
    [12, 64, 4096] (non-finite values replaced by BIG).
    Computed with jax on CPU so rounding matches the reference bit-for-bit.
    """
    import jax
    import jax.numpy as jnp
    cpu = jax.devices("cpu")[0]
    ctx = jax.default_device(cpu)
    ctx.__enter__()

    V = NVIEW
    h, w = H, W
    yy, xx = jnp.meshgrid(jnp.arange(h, dtype=jnp.float32),
                          jnp.arange(w, dtype=jnp.float32), indexing='ij')
    onehm = jnp.stack([xx.reshape(-1), yy.reshape(-1), jnp.ones(HW, jnp.float32)], 0)
    K = jnp.asarray(cam_Intri).reshape(B, V, 3, 3)
    R = jnp.asarray(cam_R).reshape(B, V, 3, 3)
    T = jnp.asarray(cam_T).reshape(B, V, 3, 1)
    Aff = jnp.asarray(affine_trans).reshape(B, V, 3, 3)
    invAff = jnp.asarray(inv_affine_trans).reshape(B, V, 3, 3)
    invK = jnp.linalg.inv(K)
    ray = jnp.einsum('bvij,bvjk,kp->bvip', invK, invAff, onehm)
    deps = jnp.array([1000.0, 5000.0], jnp.float32).reshape(2, 1, 1, 1, 1)
    xg = jnp.einsum('bvji,dbvjp->dbvip', R, deps * ray[None]) + T[None]
    xcam = jnp.einsum('boij,dbcojp->dbcoip', R, xg[:, :, :, None] - T[:, None])
    xnorm = xcam / xcam[:, :, :, :, 2:3]
    M = jnp.einsum('bvij,bvjk->bvik', Aff, K)
    uv = jnp.einsum('boij,dbcojp->dbcoip', M, xnorm)
    oth = np.array([[o for o in range(V) if o != c] for c in range(V)])
    uv = uv[:, :, jnp.arange(V)[:, None], oth]
    x0, y0 = uv[0, ..., 0, :], uv[0, ..., 1, :]
    x1, y1 = uv[1, ..., 0, :], uv[1, ..., 1, :]
    kk = (y1 - y0) / (x1 - x0)
    xs = jnp.arange(w, dtype=jnp.float32)
    ysw = kk[..., None] * (xs - x0[..., None]) + y0[..., None]   # (B,V,V-1,HW,w)
    ysh = jnp.arange(h, dtype=jnp.float32)
    xsh = (ysh - y0[..., None]) / kk[..., None] + x0[..., None]  # (B,V,V-1,HW,h)

    # Reference normalizes to [-1,1] then maps back before rounding; that
    # fp round-trip shifts values by a few ulp, so replicate it exactly.
    def _round_chain(v):
        v = jnp.where(jnp.isfinite(v), v, jnp.float32(BIG))
        g = v / jnp.float32((W - 1) / 2.0) - 1.0
        return jnp.round((g + 1.0) * 0.5 * (W - 1))

    iy = np.asarray(_round_chain(ysw), np.float32)
    ix = np.asarray(_round_chain(xsh), np.float32)
    iy = iy.reshape(NPAIR, HW, W).transpose(0, 2, 1)
    ix = ix.reshape(NPAIR, HW, H).transpose(0, 2, 1)
    ctx.__exit__(None, None, None)
    return iy, ix


def _host_indices(iy, ix):
    """clamp -> bf16 index rows [12, 2(sweep), 64(t), 4096(px)]."""
    out = np.empty((NPAIR, 2, W, HW), dtype=ml_dtypes.bfloat16)
    for s, arr in enumerate((iy, ix)):
        r = np.clip(arr, -1.0, 64.0)           # invalid -> never matches iota
        r = np.where(np.isfinite(r), r, 64.0)  # NaN paranoia
        out[:, s] = r.astype(ml_dtypes.bfloat16)
    return out


def _host_tables(heatmaps):
    """Block-diagonal bf16 two-part gather tables.

    Returns [12, 2, 32, 128, 64] bf16:
      cols  0:32  = MM1 = [even-t hi | odd-t hi]   (block-diagonal)
      cols 32:64  = MM2 = [even-t lo | odd-t lo]   (block-diagonal)
    Rows 0:64 are the 64-entry table for the even t, rows 64:128 odd t.
    x-sweep table entry (y, t): hm[o, ch, y, t];  y-sweep (x, t): hm[o, ch, t, x].
    """
    hm = np.asarray(heatmaps, np.float32).reshape(NVIEW, C, H, W)
    hi = hm.astype(ml_dtypes.bfloat16)
    lo32 = hm - hi.astype(np.float32)
    lo = lo32.astype(ml_dtypes.bfloat16)
    # residual after hi+lo is < 2^-16 relative; dropped (see module docstring)

    tab = np.zeros((NPAIR, 2, NTP, 128, 64), dtype=ml_dtypes.bfloat16)
    for p, (c, o) in enumerate(_PAIRS):
        for part, src in ((0, hi), (1, lo)):
            base = 32 * part
            # x-sweep: entry k=y, column t -> src[o, ch, y, t]; [t, y, ch]
            xs = src[o].transpose(2, 1, 0)
            tab[p, 0, :, 0:64, base + 0:base + 16] = xs[0::2]
            tab[p, 0, :, 64:128, base + 16:base + 32] = xs[1::2]
            # y-sweep: entry k=x, sweep param t'=row -> src[o, ch, t', x]
            ys = src[o].transpose(1, 2, 0)      # [t'(row), x(entry), ch]
            tab[p, 1, :, 0:64, base + 0:base + 16] = ys[0::2]
            tab[p, 1, :, 64:128, base + 16:base + 32] = ys[1::2]
    return tab


_COMPILED = {}


def _build_program():
    import concourse.bass as bass
    import concourse.bacc as bacc
    import concourse.mybir as mybir
    import concourse.tile as tile
    from contextlib import ExitStack

    dt = mybir.dt
    ops = mybir.AluOpType
    act = mybir.ActivationFunctionType

    nc = bacc.Bacc("TRN2", target_bir_lowering=False, debug=False,
                   num_devices=NCORE)

    idx_d = nc.dram_tensor("idxb", [NPAIR, 2, 2, NTP * PXS], dt.bfloat16,
                           kind="ExternalInput")
    tab_d = nc.dram_tensor("tab", [NPAIR, 2, NTP, 128, 64], dt.bfloat16,
                           kind="ExternalInput")
    ind_d = nc.dram_tensor("ind", [2, 128], dt.bfloat16, kind="ExternalInput")
    iota_d = nc.dram_tensor("iota", [128, 1], dt.float32, kind="ExternalInput")
    niota_d = nc.dram_tensor("niota", [128, 1], dt.float32, kind="ExternalInput")
    out_d = nc.dram_tensor("out", [NPAIR, 16, PXS], dt.float32,
                           kind="ExternalOutput")

    ACT_FRAC = 11  # of 32 t-pairs per (pair, sweep), handled on ScalarE

    with tile.TileContext(nc) as tc:
        with ExitStack() as ctx:
            cpool = ctx.enter_context(tc.tile_pool(name="const", bufs=1))
            tpool = ctx.enter_context(tc.tile_pool(name="tabs", bufs=2))
            ipool = ctx.enter_context(tc.tile_pool(name="idx", bufs=2))
            mpool = ctx.enter_context(tc.tile_pool(name="mask", bufs=4))
            spool = ctx.enter_context(tc.tile_pool(name="sq", bufs=3))
            apool = ctx.enter_context(tc.tile_pool(name="acc", bufs=3))
            fpool = ctx.enter_context(tc.tile_pool(name="fold", bufs=3))
            rpool = ctx.enter_context(tc.tile_pool(name="res", bufs=3))
            ppool = ctx.enter_context(tc.tile_pool(name="P", bufs=3, space="PSUM"))
            opool = ctx.enter_context(tc.tile_pool(name="O", bufs=2, space="PSUM"))

            ind = cpool.tile([2, 128], dt.bfloat16, tag="ind")
            iota = cpool.tile([128, 1], dt.float32, tag="iota")
            niota = cpool.tile([128, 1], dt.float32, tag="niota")
            nc.sync.dma_start(ind[:], ind_d.ap())
            nc.sync.dma_start(iota[:], iota_d.ap())
            nc.sync.dma_start(niota[:], niota_d.ap())

            for p in range(NPAIR):
                res_ps = None
                for s in range(2):
                    tab = tpool.tile([128, NTP * 64], dt.bfloat16, tag="tab")
                    nc.sync.dma_start(
                        tab[:].rearrange("k (g x) -> k g x", g=NTP),
                        tab_d.ap()[p, s].rearrange("g k x -> k g x"))
                    idxt = ipool.tile([2, NTP * PXS], dt.bfloat16, tag="idx")
                    nc.sync.dma_start(idxt[:], idx_d.ap()[p, s])

                    acc = apool.tile([128, PXS], dt.float32, tag="acc")
                    for gg in range(NTP // 4):
                        ops_ps = opool.tile([128, PXS], dt.float32, tag="O")
                        for slot in range(4):
                            g = gg * 4 + slot
                            P = ppool.tile([128, PXS], dt.float32, tag="P")
                            nc.tensor.matmul(
                                P[:], ind[:],
                                idxt[:, g * PXS:(g + 1) * PXS],
                                start=True, stop=True)
                            mask = mpool.tile([128, PXS], dt.bfloat16, tag="m")
                            if g % NTP < ACT_FRAC:
                                sq = spool.tile([128, PXS], dt.bfloat16, tag="sq")
                                nc.scalar.activation(sq[:], P[:], act.Square,
                                                     bias=niota[:], scale=1.0)
                                nc.scalar.activation(mask[:], sq[:], act.Relu,
                                                     bias=1.0, scale=-1.0)
                            else:
                                nc.vector.tensor_scalar(mask[:], P[:], iota[:],
                                                        None, ops.is_equal)
                            tslice = tab[:, g * 64:g * 64 + 32]
                            nc.tensor.matmul(
                                ops_ps[32 * slot:32 * slot + 32, :],
                                tslice, mask[:], start=True, stop=False,
                                tile_position=(0, 32 * slot))
                            tslice2 = tab[:, g * 64 + 32:g * 64 + 64]
                            nc.tensor.matmul(
                                ops_ps[32 * slot:32 * slot + 32, :],
                                tslice2, mask[:], start=False, stop=True,
                                tile_position=(0, 32 * slot))
                        if gg == 0:
                            nc.vector.tensor_copy(acc[:], ops_ps[:])
                        else:
                            nc.vector.tensor_tensor(acc[:], acc[:], ops_ps[:],
                                                    ops.max)
                    # fold 4 col-group slots (partition blocks of 32)
                    f64 = fpool.tile([64, PXS], dt.float32, tag="f64")
                    nc.sync.dma_start(f64[:], acc[64:128, :])
                    nc.vector.tensor_tensor(f64[:], f64[:], acc[0:64, :], ops.max)
                    f32t = fpool.tile([32, PXS], dt.float32, tag="f32")
                    nc.sync.dma_start(f32t[:], f64[32:64, :])
                    nc.vector.tensor_tensor(f32t[:], f32t[:], f64[0:32, :], ops.max)
                    # fold t-parity (partition blocks of 16)
                    f16 = fpool.tile([16, PXS], dt.float32, tag="f16")
                    nc.sync.dma_start(f16[:], f32t[16:32, :])
                    nc.vector.tensor_tensor(f16[:], f16[:], f32t[0:16, :], ops.max)
                    if s == 0:
                        res_ps = rpool.tile([16, PXS], dt.float32, tag="res")
                        nc.vector.tensor_copy(res_ps[:], f16[:])
                    else:
                        nc.vector.tensor_tensor(res_ps[:], res_ps[:], f16[:],
                                                ops.max)
                nc.sync.dma_start(out_d.ap()[p], res_ps[:])

    nc.compile()
    return nc


def _make_in_maps(inputs):
    ysw, xsh = _line_coords(inputs["affine_trans"], inputs["cam_Intri"],
                            inputs["cam_R"], inputs["cam_T"],
                            inputs["inv_affine_trans"])
    idx = _host_indices(ysw, xsh)          # [12, 2, 64, 4096] bf16
    tab = _host_tables(inputs["heatmaps"])  # [12, 2, 32, 128, 64] bf16

    ind = np.zeros((2, 128), dtype=ml_dtypes.bfloat16)
    ind[0, 0:64] = 1.0
    ind[1, 64:128] = 1.0
    iota = (np.arange(128, dtype=np.float32) % 64).reshape(128, 1)
    niota = np.ascontiguousarray(-iota)

    in_maps = []
    for i in range(NCORE):
        # pixel shard: image rows 8i..8i+8 -> px slice in HW-flattened order
        sl = slice(i * PXS, (i + 1) * PXS)
        idx_i = idx[:, :, :, sl]                       # [12, 2, 64, 512]
        # device layout [pair, sweep, parity, g*512+px]
        idxb = np.ascontiguousarray(
            idx_i.reshape(NPAIR, 2, NTP, 2, PXS).transpose(0, 1, 3, 2, 4)
        ).reshape(NPAIR, 2, 2, NTP * PXS)
        in_maps.append({"idxb": idxb, "tab": tab, "ind": ind,
                        "iota": iota, "niota": niota})
    return in_maps


def kernel(heatmaps, affine_trans, cam_Intri, cam_R, cam_T, inv_affine_trans):
    from concourse.bass_utils import run_bass_kernel_spmd

    heatmaps = np.asarray(heatmaps)
    in_dtype = heatmaps.dtype
    inputs = {"heatmaps": heatmaps, "affine_trans": affine_trans,
              "cam_Intri": cam_Intri, "cam_R": cam_R, "cam_T": cam_T,
              "inv_affine_trans": inv_affine_trans}

    if "prog" not in _COMPILED:
        _COMPILED["prog"] = _build_program()
    nc = _COMPILED["prog"]

    in_maps = _make_in_maps(inputs)
    res = run_bass_kernel_spmd(nc, in_maps, list(range(NCORE)))

    out = np.empty((NVIEW, NVIEW - 1, C, H, W), dtype=np.float32)
    for i in range(NCORE):
        o_i = res.results[i]["out"].reshape(NPAIR, C, ROWS, W)
        for p, (c, o) in enumerate(_PAIRS):
            slot = [v for v in range(NVIEW) if v != c].index(o)
            out[c, slot, :, i * ROWS:(i + 1) * ROWS, :] = o_i[p]
    return out.reshape(NVIEW, NVIEW - 1, C, H, W).astype(in_dtype, copy=False)
